# revision 7
# baseline (speedup 1.0000x reference)
"""Causal self-attention (B=4, T=2048, D=1024, H=16) on 8 trn2 NeuronCores.

Sharding: data-parallel over batch (4) x tensor-parallel over heads (2 groups
of 8 heads). Core c handles batch c//2 and head-group c%2. Each core:
  1. qkv projection for its 512 qkv columns (8 heads x 64 x {q,k,v})
  2. causal attention for its 8 heads (flash-style, transposed S^T tiles,
     unnormalized exp + ones-column row sums, normalize at the end)
  3. partial out-projection  y_local @ w_out[rows of its heads]
Host sums the two head-group partials per batch (the "all-reduce").

Matmul inputs are float32r-typed (full-rate fp32 mode on the PE array);
walrus requires every producer of an f32r matmul operand to emit f32r.
"""

import numpy as np

import concourse.bass as bass
import concourse.mybir as mybir
from concourse import bacc
from concourse.tile import TileContext
from concourse.bass_utils import run_bass_kernel_spmd

F32 = mybir.dt.float32
P = 128

B, T, D, H, HD = 4, 2048, 1024, 16, 64
HLOC = H // 2          # heads per core
W = HLOC * HD          # 512: local qkv width per section
N_CORES = 8


def build_nc(T=T, D=D, mm_dtype=mybir.dt.float32r):
    S_D = D // P           # contraction slices (8)
    NPAIR = HLOC // 2      # head pairs (4)
    TT = T // P            # 128-row tiles (16)
    IC = 512               # i-chunk (moving free dim)
    NIC = T // IC          # 4
    JPC = IC // P          # j-tiles per i-chunk (4)
    NEC = D // 512         # out-proj column chunks (2)
    scale = float(1.0 / np.sqrt(HD))
    MMD = mm_dtype

    nc = bacc.Bacc("TRN2", target_bir_lowering=False, debug=False,
                   num_devices=N_CORES)

    def mm(out, lhsT, rhs, start, stop):
        nc.tensor.matmul(out, lhsT=lhsT, rhs=rhs, start=start, stop=stop)

    xT = nc.declare_dram_parameter("xT", [D, T], F32, isOutput=False)
    wq = nc.declare_dram_parameter("wq", [D, W], F32, isOutput=False)
    wk = nc.declare_dram_parameter("wk", [D, W], F32, isOutput=False)
    wv = nc.declare_dram_parameter("wv", [D, W], F32, isOutput=False)
    wo = nc.declare_dram_parameter("wo", [W, D], F32, isOutput=False)
    out = nc.declare_dram_parameter("out", [T, D], F32, isOutput=True)

    xT_r = xT.rearrange("(s p) t -> p s t", p=P).bitcast(MMD)   # [128, S_D, T]
    wq_r = wq.rearrange("(s p) n -> p s n", p=P).bitcast(MMD)   # [128, S_D, W]
    wk_r = wk.rearrange("(s p) n -> p s n", p=P).bitcast(MMD)
    wv_r = wv.rearrange("(s p) n -> p s n", p=P).bitcast(MMD)
    wo_r = wo.rearrange("(m p) e -> p m e", p=P).bitcast(MMD)   # [128, NPAIR, D]
    out_r = out.rearrange("(t p) e -> p t e", p=P)              # [128, TT, D]

    with TileContext(nc) as tc:
        with (
            tc.tile_pool(name="const", bufs=1) as const_pool,
            tc.tile_pool(name="persist", bufs=1) as persist,
        ):
            # diagonal causal masks: mask[p, r, f] = 1.0 if p + r*128 <= f else 0
            # (mask feeds only DVE multiplies, so plain f32; the f32r ones
            # tiles are produced via DVE cast-copies since memset can't
            # emit f32r)
            mask_sb = const_pool.tile([P, JPC, IC], F32)
            ones_f32 = const_pool.tile([P, max(TT * HLOC, HD)], F32)
            nc.gpsimd.memset(ones_f32[:], 1.0)
            ones64 = const_pool.tile([1, HD], MMD)
            nc.vector.tensor_copy(ones64[:], ones_f32[0:1, 0:HD])
            for r in range(JPC):
                nc.gpsimd.memset(mask_sb[:, r, :], 1.0)
                nc.gpsimd.affine_select(
                    out=mask_sb[:, r, :], in_=mask_sb[:, r, :],
                    compare_op=mybir.AluOpType.is_ge, fill=0.0,
                    base=-(r * P), pattern=[[1, IC]], channel_multiplier=-1,
                )

            # persistent activations: q^T, k^T as [pair-row, pair, T];
            # v natural per (tile, head) with an appended ones column
            qT = persist.tile([P, NPAIR, T], MMD)
            kT = persist.tile([P, NPAIR, T], MMD)
            v_sb = persist.tile([P, TT, HLOC, HD + 1], MMD)
            nc.vector.tensor_copy(
                v_sb[:, :, :, HD:HD + 1],
                ones_f32[:, 0:TT * HLOC].rearrange(
                    "p (a b) -> p a b", b=HLOC)[:, :, :, None])

            # ---------------- phase 1: qkv projection ----------------
            with (
                tc.tile_pool(name="w1", bufs=1) as w1_pool,
                tc.tile_pool(name="xp", bufs=2) as x_pool,
                tc.tile_pool(name="ps1", bufs=4, space="PSUM") as ps1,
            ):
                wq_sb = w1_pool.tile([P, S_D, W], MMD)
                wk_sb = w1_pool.tile([P, S_D, W], MMD)
                wv_sb = w1_pool.tile([P, S_D, W], MMD)
                nc.sync.dma_start(wq_sb[:], wq_r)
                nc.sync.dma_start(wk_sb[:], wk_r)
                nc.sync.dma_start(wv_sb[:], wv_r)

                for ic in range(NIC):
                    xc = x_pool.tile([P, S_D, IC], MMD)
                    nc.sync.dma_start(xc[:], xT_r[:, :, ic * IC:(ic + 1) * IC])
                    for m in range(NPAIR):  # q^T pairs
                        ps = ps1.tile([P, IC], F32, tag="ps1")
                        for s in range(S_D):
                            mm(ps, wq_sb[:, s, m * P:(m + 1) * P], xc[:, s, :],
                               s == 0, s == S_D - 1)
                        nc.vector.tensor_copy(qT[:, m, ic * IC:(ic + 1) * IC], ps)
                    for m in range(NPAIR):  # k^T pairs
                        ps = ps1.tile([P, IC], F32, tag="ps1")
                        for s in range(S_D):
                            mm(ps, wk_sb[:, s, m * P:(m + 1) * P], xc[:, s, :],
                               s == 0, s == S_D - 1)
                        nc.vector.tensor_copy(kT[:, m, ic * IC:(ic + 1) * IC], ps)
                    for itl in range(JPC):  # v natural tiles
                        tt = ic * JPC + itl
                        ps = ps1.tile([P, IC], F32, tag="ps1")
                        for s in range(S_D):
                            mm(ps, xc[:, s, itl * P:(itl + 1) * P], wv_sb[:, s, :],
                               s == 0, s == S_D - 1)
                        nc.vector.tensor_copy(
                            v_sb[:, tt, :, 0:HD],
                            ps.rearrange("p (h d) -> p h d", d=HD))

            # ---------------- phase 2 + 3 ----------------
            with (
                tc.tile_pool(name="w2", bufs=1) as w2_pool,
                tc.tile_pool(name="ptp", bufs=3) as pt_pool,
                tc.tile_pool(name="recp", bufs=2) as rec_pool,
                tc.tile_pool(name="recbp", bufs=2) as recb_pool,
                tc.tile_pool(name="outp", bufs=3) as out_pool,
                tc.tile_pool(name="ps_s", bufs=3, space="PSUM") as psum_s,
                tc.tile_pool(name="ps_y", bufs=2, space="PSUM") as psum_y,
                tc.tile_pool(name="ps_r", bufs=2, space="PSUM") as psum_r,
            ):
                wo_sb = w2_pool.tile([P, NPAIR, D], MMD)
                nc.sync.dma_start(wo_sb[:], wo_r)
                yT = w2_pool.tile([P, NPAIR, T], MMD)

                # attention, software-pipelined: emit S(jb+1)/exp(jb+1)
                # before PV(jb) so the PE never waits on the ACT exp
                for h in range(HLOC):
                    m, po = h // 2, (h % 2) * HD
                    for ic in range(NIC):
                        njb = JPC * (ic + 1)
                        ps_y = psum_y.tile([HD + 1, IC], F32, tag="psy")
                        pts = []
                        for jb in range(njb):
                            ps_s = psum_s.tile([P, IC], F32, tag="pss")
                            mm(ps_s, kT[po:po + HD, m, jb * P:(jb + 1) * P],
                               qT[po:po + HD, m, ic * IC:(ic + 1) * IC],
                               True, True)
                            pt = pt_pool.tile([P, IC], MMD, tag="pt")
                            nc.scalar.activation(
                                pt[:], ps_s, mybir.ActivationFunctionType.Exp,
                                scale=scale)
                            r = jb - JPC * ic
                            if r >= 0:
                                nc.vector.tensor_mul(pt[:], pt[:], mask_sb[:, r, :])
                            pts.append(pt)
                            if len(pts) > 1:  # PV for previous j-tile
                                mm(ps_y, v_sb[:, jb - 1, h, :], pts[-2],
                                   jb - 1 == 0, False)
                        mm(ps_y, v_sb[:, njb - 1, h, :], pts[-1],
                           njb - 1 == 0, True)
                        # normalize: y^T[d, i] * (1/sum[i])
                        rec = rec_pool.tile([1, IC], MMD, tag="rec")
                        with nc.allow_low_precision(
                                reason="f32r rounding for matmul broadcast"):
                            nc.vector.reciprocal(rec[:], ps_y[HD:HD + 1, :])
                        ps_rec = psum_r.tile([HD, IC], F32, tag="psr")
                        mm(ps_rec, ones64[:], rec[:], True, True)
                        recb = recb_pool.tile([HD, IC], F32, tag="recb")
                        nc.vector.tensor_copy(recb[:], ps_rec)
                        nc.vector.tensor_mul(
                            yT[po:po + HD, m, ic * IC:(ic + 1) * IC],
                            ps_y[0:HD, :], recb[:])

                # out projection (partial over this core's heads)
                for it in range(TT):
                    for ec in range(NEC):
                        ps_o = psum_s.tile([P, 512], F32, tag="pss")
                        for m in range(NPAIR):
                            mm(ps_o, yT[:, m, it * P:(it + 1) * P],
                               wo_sb[:, m, ec * 512:(ec + 1) * 512],
                               m == 0, m == NPAIR - 1)
                        o_t = out_pool.tile([P, 512], F32, tag="ot")
                        nc.vector.tensor_copy(o_t[:], ps_o)
                        nc.sync.dma_start(out_r[:, it, ec * 512:(ec + 1) * 512],
                                          o_t[:])

    nc.compile()
    return nc


def shard_inputs(x, w_qkv, w_out):
    """Full inputs -> list of 8 per-core input maps."""
    in_maps = []
    for c in range(N_CORES):
        b, g = c // 2, c % 2
        hsl = slice(g * W, (g + 1) * W)
        in_maps.append({
            "xT": np.ascontiguousarray(x[b].T),
            "wq": np.ascontiguousarray(w_qkv[:, 0 * D:1 * D][:, hsl]),
            "wk": np.ascontiguousarray(w_qkv[:, 1 * D:2 * D][:, hsl]),
            "wv": np.ascontiguousarray(w_qkv[:, 2 * D:3 * D][:, hsl]),
            "wo": np.ascontiguousarray(w_out[hsl, :]),
        })
    return in_maps


_NC_CACHE = {}


def kernel(x, w_qkv, w_out, **run_kwargs):
    x = np.asarray(x, dtype=np.float32)
    w_qkv = np.asarray(w_qkv, dtype=np.float32)
    w_out = np.asarray(w_out, dtype=np.float32)
    if "nc" not in _NC_CACHE:
        _NC_CACHE["nc"] = build_nc()
    nc = _NC_CACHE["nc"]
    in_maps = shard_inputs(x, w_qkv, w_out)
    res = run_bass_kernel_spmd(nc, in_maps, core_ids=list(range(N_CORES)),
                               **run_kwargs)
    _NC_CACHE["last_results"] = res
    outs = [r["out"] for r in res.results]
    full = np.stack([outs[2 * b] + outs[2 * b + 1] for b in range(B)], axis=0)
    return full


if __name__ == "__main__":
    rng = np.random.default_rng(0)
    x = rng.standard_normal((B, T, D), dtype=np.float32)
    w_qkv = (rng.standard_normal((D, 3 * D), dtype=np.float32) / np.sqrt(D))
    w_out = (rng.standard_normal((D, D), dtype=np.float32) / np.sqrt(D))
    y = kernel(x, w_qkv, w_out)
    print("out", y.shape, y.dtype, float(np.abs(y).mean()))
